# revision 31
# baseline (speedup 1.0000x reference)
"""Cosine-similarity (pairwise, normalized by sqrt(|a||b|)+eps) Trainium2 kernel.

Problem: first_vector [8192, 512] f32, second_vector [8192, 512] f32,
output sim [8192, 8192] f32 with
    sim = (A @ B.T) / (sqrt(|A_n| * |B_m|) + 1e-6)        (normalize=1)

Strategy (8 NeuronCores, SPMD, no collectives):
  * 2D shard: 4-way over A rows x 2-way over B rows. Core c=(ni,mj)
    computes the [2048, 4096] output slab at (ni*2048, mj*4096).
  * All matmul operands are packed HOST-side into d-major (transposed)
    tiled layouts, so the PE does nothing but the 512 GEMM matmuls
    (215.8ns each at the fp16 roofline, measured) plus 8 cheap
    sum-of-squares matmuls. The previous design transposed on-device
    via identity matmuls (+15us PE) and burned ~55us of DVE/ACT on
    transpose evacuations + operand scaling.
  * Normalization is separable (eps shifts the result by <1e-7 rel):
    scale A rows by ssqA^-1/4 and B rows by ssqB^-1/4.
      - A ssq: an auxiliary ROW-major copy of A is loaded (+2MB DMA);
        square-with-accum_out gives ssq in column form [128,1] per
        row-tile - exactly the per-partition scalar an evacuation wants.
      - B ssq: square+add the d-major tiles, then one matmul against an
        all-ones [128,128] lhsT: out[p,f] = sum_k s1[k,f] for every p -
        reduce AND broadcast in one ~215ns PE op (the probed gpsimd
        partition_all_reduce costs 3537ns).
      - chains run reciprocal_approx_fast (DVE, ~51 ULP, straight off
        PSUM for B) then sqrt+sqrt on ACT, landing sbB in f16
        broadcast form / saT in f32 column form.
  * Evacuations (64 x [128,2,512] PSUM f32 -> f16) all run as ONE DVE
    scalar_tensor_tensor per slot on RAW B operands:
    out = (psum * saT[t]) * sbB[2p:2p+2] (~1300ns, barely above a plain
    cast). No operand pre-scaling exists at all, so no B-scale work ever
    gates a GEMM slot, and GpSimd multiplies never contend with PE SBUF
    reads (pre-scaling B on GpSimd measured SLOWER end-to-end).
    Evacuations are emitted 2 slots behind the GEMM so chain producers
    precede consumers in each engine's stream; PSUM holds 3 slots + the
    ssq/warmup bank.
  * The first four slots are single-B-group (t-pair x one s): their
    evacuations depend on only ONE sbB chain (b0's lands ~4.5us before
    b1's) which beats the PSUM-recycle deadline while bd1 still streams.
  * Startup is DMA-stream-bound (~0.27MB/us effective): transfers are
    batched (each dma_start costs ~0.6us of Sync issue time) and
    ordered by need-time; nothing issues until the ~6us instruction
    -load prolog ends (excluded from the reported exec time).
  * The PE clock governor is ACTIVITY-driven: 16 back-to-back warm-up
    matmuls during the input-DMA wait pull the 0.65 -> 2.4GHz ramp
    completion from ~28us down to ~12us (measured via HAM track).
    Note the device can also pin itself to 2.0GHz under sustained
    benching load (259ns steady matmuls instead of 215.8ns) - run-to-
    run variance of several us is thermal, not structural.
  * The ~9.5us exit protocol (289 EVENT_SEMAPHORE barrier events) and
    ~6us instruction-load prolog are fixed costs, identical even for a
    5-op probe kernel.
  * fp16 everywhere off-chip: 8MB in, 16.8MB out per core, against a
    ~112us PE window - DMA never binds after the startup phase.
"""

import numpy as np

_N, _M, _D = 8192, 8192, 512
_P = 128
_GRID_N, _GRID_M = 4, 2
_AN = _N // _GRID_N        # A rows per core (2048)
_BM = _M // _GRID_M        # B rows per core (4096)
_KC = _D // _P             # contraction chunks (4)
_NS = 512                  # moving free dim per matmul (one PSUM bank of f32)

TRACE = False              # test harness sets True to collect an NTFF profile
LAST_RESULTS = None        # BassKernelResults of the last run (for test.py)

_NC_CACHE = {}


def _build_nc(normalize: bool):
    import concourse.bass as bass
    import concourse.mybir as mybir
    import concourse.tile as tile
    from concourse import bacc

    f32 = mybir.dt.float32
    f16 = mybir.dt.float16
    nc = bacc.Bacc("TRN2", target_bir_lowering=False, debug=False,
                   enable_asserts=False)

    KA = _AN // _P             # 16 A row-tiles (8 pairs)
    NSC = _BM // _NS           # 8 B column groups of 512
    NG = KA // 2               # 8 A pairs
    SQ = mybir.ActivationFunctionType.Square
    CP = mybir.ActivationFunctionType.Copy
    MUL = mybir.AluOpType.mult
    ADD = mybir.AluOpType.add

    # d-major A pairs: row (g*128+p), col (h, kc, r) = A[256g+128h+r, 128kc+p]
    ad_d = nc.declare_dram_parameter("ad", [NG * _P, 2 * _KC * _P], f16,
                                     isOutput=False)
    # d-major B groups: row (s*128+p), col (kc, c) = B[512s+c, 128kc+p]
    bd_d = nc.declare_dram_parameter("bd", [NSC * _P, _KC * _NS], f16,
                                     isOutput=False)
    # row-major A pairs (aux, for ssq): row (g*128+p), col (h, d)
    aa_d = nc.declare_dram_parameter("aa", [NG * _P, 2 * _D], f16,
                                     isOutput=False)
    out_d = nc.declare_dram_parameter("out", [_AN, _BM], f16, isOutput=True)

    # Greedy DVE/ACT balance for the sa-only evacuations (ns, measured).
    load = {"dve": 0.0, "act": 0.0}

    with tile.TileContext(nc) as tc:
        with (
            tc.tile_pool(name="const", bufs=1) as const_pool,
            tc.tile_pool(name="persist", bufs=1) as persist,
            tc.tile_pool(name="sqp", bufs=2) as sqp,
            tc.tile_pool(name="s1p", bufs=2) as s1p,
            tc.tile_pool(name="chp", bufs=2) as chp,
            tc.tile_pool(name="dmp", bufs=2) as dmp,
            tc.tile_pool(name="ostp", bufs=8) as ostp,
            tc.tile_pool(name="mpsum", bufs=3, space=bass.MemorySpace.PSUM) as mpsum,
            tc.tile_pool(name="ssqp", bufs=2, space=bass.MemorySpace.PSUM) as ssqp,
        ):
            # Memsets first so every engine's first data arrives ASAP.
            wsrc = const_pool.tile([_P, _NS], f16)
            nc.vector.memset(wsrc[:], 0.5)
            ones = const_pool.tile([_P, _P], f16)
            nc.vector.memset(ones[:], 1.0)
            dsrc = const_pool.tile([_P, 1], f32)
            nc.vector.memset(dsrc[:], 1.0)

            # ACT table preloads on dependency-free data (Sqrt is needed
            # first, by the chains; Square by the A/B squares).
            ddst = const_pool.tile([_P, 1], f32)
            nc.scalar.sqrt(ddst[:], dsrc[:])
            nc.scalar.activation(ddst[:], dsrc[:], SQ)

            # Consolidated operand tiles (single allocations; sub-tile
            # DMAs + sub-tile reads use Tile's range-based deps).
            adT = persist.tile([_P, NG, 2, _KC, _P], f16, name="adT", tag="adT")
            bdT = persist.tile([_P, NSC, _KC, _NS], f16, name="bdT", tag="bdT")
            aaT = persist.tile([_P, NG, 2, _D], f16, name="aaT", tag="aaT")

            ssqA = persist.tile([_P, KA], f32, name="ssqA", tag="ssqA")
            shA = persist.tile([_P, KA], f32, name="shA", tag="shA")
            saT = persist.tile([_P, KA], f32, name="saT", tag="saT")
            sbB = persist.tile([_P, NSC, _NS], f16, name="sbB", tag="sbB")

            # Input DMA: each dma_start costs ~0.6us of Sync-sequencer
            # issue time, so batch multi-pair transfers; first-needed
            # tiles go individually, in need order.
            def dma_bd(s0, s1_):
                n = s1_ - s0
                nc.sync.dma_start(
                    bdT[:, s0:s1_],
                    bd_d[s0 * _P:s1_ * _P, :]
                    .rearrange("(s p) (k c) -> p s k c", s=n, k=_KC))

            def dma_ad(g0, g1):
                n = g1 - g0
                nc.sync.dma_start(
                    adT[:, g0:g1],
                    ad_d[g0 * _P:g1 * _P, :]
                    .rearrange("(g p) (h k r) -> p g h k r", g=n, h=2, k=_KC))

            def dma_aa(g0, g1):
                n = g1 - g0
                nc.sync.dma_start(
                    aaT[:, g0:g1],
                    aa_d[g0 * _P:g1 * _P, :]
                    .rearrange("(g p) (h d) -> p g h d", g=n, h=2))

            # Arrival model: transfers stream at ~0.28MB/us starting ~2us
            # after the prolog; order by need-time (first evac needs aa01;
            # the PE ramp makes early GEMM slots ~2x long).
            dma_bd(0, 1)
            dma_ad(0, 1)
            dma_aa(0, 2)
            dma_ad(1, 2)
            dma_bd(1, 2)
            dma_ad(2, 4)
            dma_aa(2, 4)
            dma_aa(4, 6)
            dma_bd(2, 4)
            dma_aa(6, 8)
            dma_ad(4, 8)
            dma_bd(4, 8)

            # Warm the PE clock during the input-DMA wait (~5us until bd0
            # and ad0 land). The clock governor appears activity-driven:
            # runs whose PE idles early reach the full 2.4GHz pstate
            # several us later, so keep the PE busy back-to-back.
            for _ in range(8):
                warm = ssqp.tile([_P, _NS], f32, tag="ssqp")
                for _ in range(2):
                    nc.tensor.matmul(warm[:], lhsT=ones[:], rhs=wsrc[:],
                                     start=True, stop=True)

            def b_squares(s, engine, adds):
                """Squares + adds of d-major B group s -> s1 [128, 512]."""
                sq = sqp.tile([_P, _KC, _NS], f16, tag="sq")
                for j in range(2):
                    if engine == "dve":
                        nc.vector.tensor_tensor(sq[:, 2 * j:2 * j + 2],
                                                bdT[:, s, 2 * j:2 * j + 2],
                                                bdT[:, s, 2 * j:2 * j + 2],
                                                op=MUL)
                        load["dve"] += 1380
                    else:
                        nc.scalar.activation(sq[:, 2 * j:2 * j + 2],
                                             bdT[:, s, 2 * j:2 * j + 2], SQ)
                        load["act"] += 1060
                s2 = s1p.tile([_P, 2, _NS], f16, tag="s2")
                s1 = s1p.tile([_P, _NS], f16, tag="s1")
                eng = nc.vector if adds == "dve" else nc.gpsimd
                eng.tensor_tensor(s2[:], sq[:, 0:2], sq[:, 2:4], op=ADD)
                eng.tensor_tensor(s1[:], s2[:, 0], s2[:, 1], op=ADD)
                if adds == "dve":
                    load["dve"] += 1900
                return s1

            def b_ssq_mm(s, s1):
                """All-ones matmul: ssq_b reduced + broadcast into PSUM."""
                ps = ssqp.tile([_P, _NS], f32, tag="ssqp")
                nc.tensor.matmul(ps[:], lhsT=ones[:], rhs=s1[:], start=True,
                                 stop=True)
                return ps

            def b_chain(s, ps):
                """sbB[s] = ssq^-1/4: reciprocal_approx_fast (DVE, reads
                PSUM) then sqrt+sqrt (ACT, f16 out)."""
                rec = chp.tile([_P, _NS], f32, tag="recb")
                nc.vector.reciprocal_approx_fast(rec[:], ps[:])
                nc.scalar.sqrt(rec[:], rec[:])
                nc.scalar.sqrt(sbB[:, s, :], rec[:])
                load["dve"] += 800
                load["act"] += 1440

            def a_ssq(g, engine):
                """ssq of A pair g (column form) + chain into saT."""
                for h in range(2):
                    t = 2 * g + h
                    dump = dmp.tile([_P, _D], f16, tag="dump")
                    if engine == "dve":
                        nc.vector.scalar_tensor_tensor(
                            dump[:], aaT[:, g, h], 1.0, aaT[:, g, h],
                            op0=MUL, op1=MUL, accum_out=ssqA[:, t:t + 1])
                        load["dve"] += 700
                    else:
                        nc.scalar.activation(dump[:], aaT[:, g, h], SQ,
                                             accum_out=ssqA[:, t:t + 1])
                        load["act"] += 800
                c = slice(2 * g, 2 * g + 2)
                nc.scalar.sqrt(shA[:, c], ssqA[:, c])
                nc.scalar.sqrt(shA[:, c], shA[:, c])
                nc.vector.reciprocal(saT[:, c], shA[:, c])
                load["act"] += 400
                load["dve"] += 120

            # Pre-loop prep: B0/B1 squares on DVE (startup critical);
            # A pair 0's ACT half fills ACT's idle window before the B
            # chains' sqrts queue up.
            if normalize:
                s1_b = {0: b_squares(0, "dve", "dve"),
                        1: b_squares(1, "dve", "dve")}
                ps_b = {}
                a_ssq(0, "dve")

            # Deferred-evacuation pipeline: evac for slot i is emitted
            # alongside slot i+2 so chain producers precede consumers in
            # every engine's program order. mpsum holds 3 slots.
            pending = []

            def emit_evac():
                kind, p, t, ps2 = pending.pop(0)
                ost = ostp.tile([_P, 2, _NS], f16, tag="ost")
                if kind == "split":
                    # Single-B-group startup slot: bank h holds row-tile
                    # (2p... here p is the t-pair index) x group t(=s).
                    tp, s = p, t
                    for h in range(2):
                        tt = 2 * tp + h
                        nc.vector.scalar_tensor_tensor(
                            ost[:, h], ps2[:, h], saT[:, tt:tt + 1],
                            sbB[:, s, :], op0=MUL, op1=MUL)
                        load["dve"] += 700
                        nc.sync.dma_start(
                            out_d[tt * _P:(tt + 1) * _P,
                                  s * _NS:(s + 1) * _NS],
                            ost[:, h],
                        )
                    return
                if not normalize:
                    if load["dve"] <= load["act"]:
                        nc.vector.tensor_copy(ost[:], ps2[:])
                        load["dve"] += 1230
                    else:
                        nc.scalar.copy(ost[:], ps2[:])
                        load["act"] += 1100
                elif p == 0:
                    # Raw B operands; both scales in one DVE op.
                    nc.vector.scalar_tensor_tensor(
                        ost[:], ps2[:], saT[:, t:t + 1], sbB[:, 0:2, :],
                        op0=MUL, op1=MUL)
                    load["dve"] += 1300
                else:
                    nc.vector.scalar_tensor_tensor(
                        ost[:], ps2[:], saT[:, t:t + 1],
                        sbB[:, 2 * p:2 * p + 2, :], op0=MUL, op1=MUL)
                    load["dve"] += 1300
                nc.sync.dma_start(
                    out_d[t * _P:(t + 1) * _P,
                          2 * p * _NS:(2 * p + 2) * _NS],
                    ost[:].rearrange("m h f -> m (h f)"),
                )

            # Per-slot prep emissions for p=0. The PE ramp makes early
            # slots ~2x longer, absorbing the chain latencies; A-pair
            # ssq goes late in the ACT queue (only needed by evac t=2g).
            def prep_p0(t):
                if t == 0:
                    # Both ones-matmuls up front: slot 1's 8 GEMM matmuls
                    # must not precede ones_mm(1) in PE priority order, or
                    # the B1 chain (and the first evacuation) slips ~3us.
                    ps_b[0] = b_ssq_mm(0, s1_b[0])
                    b_chain(0, ps_b[0])
                    ps_b[1] = b_ssq_mm(1, s1_b[1])
                    b_chain(1, ps_b[1])
                elif t == 2:
                    a_ssq(1, "dve")
                elif t == 3:
                    a_ssq(2, "act")
                elif t == 7:
                    a_ssq(3, "act")
                elif 8 <= t <= 11:
                    a_ssq(t - 4, "act")    # pairs 4..7
                elif t == 14:
                    # B2/B3 prep dead last in p=0: anything earlier lets
                    # its big ACT squares cut between the final A-pair ssq
                    # chain steps (saT for the t>=12 evacuations). sbB[2:4]
                    # is only needed ~2 slots into p=1.
                    s1_b[2] = b_squares(2, "act", "pool")
                elif t == 15:
                    s1_b[3] = b_squares(3, "act", "pool")

            def prep_p(p, t):
                sa_, sb_ = 2 * p + 2, 2 * p + 3
                if p == 1:
                    if t == 0:
                        ps_b[2] = b_ssq_mm(2, s1_b[2])
                    elif t == 1:
                        ps_b[3] = b_ssq_mm(3, s1_b[3])
                        b_chain(2, ps_b[2])
                    elif t == 2:
                        b_chain(3, ps_b[3])
                if t == 3:
                    s1_b[sa_] = b_squares(sa_, "act", "pool")
                elif t == 4:
                    s1_b[sb_] = b_squares(sb_, "act", "pool")
                elif t == 5:
                    ps_b[sa_] = b_ssq_mm(sa_, s1_b[sa_])
                elif t == 6:
                    ps_b[sb_] = b_ssq_mm(sb_, s1_b[sb_])
                elif t == 7:
                    b_chain(sa_, ps_b[sa_])
                elif t == 8:
                    b_chain(sb_, ps_b[sb_])

            # GEMM stream: column-group pairs p, row-tiles t. When
            # normalizing, p=0's first four slots are single-B-group
            # (t-pair x one s): their evacuations need only ONE sbB chain
            # and the PE starts on bd0/ad0/ad1 while bd1 still streams -
            # b0's chain lands ~4.5us before b1's.
            def mm_slot(tlist_s):
                ps2 = mpsum.tile([_P, 2, _NS], f32, tag="ps2")
                for h, (t, s) in enumerate(tlist_s):
                    for k in range(_KC):
                        rhs = bdT[:, s, k, :]
                        nc.tensor.matmul(
                            ps2[:, h],
                            lhsT=adT[:, t // 2, t % 2, k, :],
                            rhs=rhs,
                            start=(k == 0),
                            stop=(k == _KC - 1),
                        )
                return ps2

            for p in range(NSC // 2):
                split_head = normalize and p == 0
                for t in range(KA):
                    if len(pending) >= 2:
                        emit_evac()

                    if split_head and t < 4:
                        tp, s = t % 2, t // 2
                        ps2 = mm_slot([(2 * tp, s), (2 * tp + 1, s)])
                        pending.append(("split", tp, s, ps2))
                    else:
                        ps2 = mm_slot([(t, 2 * p), (t, 2 * p + 1)])
                        pending.append(("pair", p, t, ps2))

                    if normalize:
                        if p == 0:
                            prep_p0(t)
                        elif p < 3:
                            prep_p(p, t)

            while pending:
                emit_evac()

    nc.compile()
    return nc


def _get_nc(normalize: bool):
    key = bool(normalize)
    if key not in _NC_CACHE:
        _NC_CACHE[key] = _build_nc(key)
    return _NC_CACHE[key]


def _pack_ad(a16):
    """[2048, 512] f16 -> d-major pair tiles [1024, 1024]:
    row (g*128+p), col (h*512 + kc*128 + r) = A[g*256+h*128+r, kc*128+p]."""
    return (a16.reshape(8, 2, _P, _KC, _P)
            .transpose(0, 4, 1, 3, 2)
            .reshape(8 * _P, 2 * _KC * _P))


def _pack_bd(b16):
    """[4096, 512] f16 -> d-major group tiles [1024, 2048]:
    row (s*128+p), col (kc*512 + c) = B[s*512+c, kc*128+p]."""
    return (b16.reshape(8, _NS, _KC, _P)
            .transpose(0, 3, 2, 1)
            .reshape(8 * _P, _KC * _NS))


def _pack_aa(a16):
    """[2048, 512] f16 row-major pairs [1024, 1024]: partition p of pair g
    holds rows g*256+p and g*256+128+p side by side."""
    return (a16.reshape(8, 2, _P, _D)
            .transpose(0, 2, 1, 3)
            .reshape(8 * _P, 2 * _D))


def kernel(first_vector, second_vector, normalize):
    global LAST_RESULTS
    from concourse.bass_utils import run_bass_kernel_spmd

    a = np.asarray(first_vector, dtype=np.float32).astype(np.float16)
    b = np.asarray(second_vector, dtype=np.float32).astype(np.float16)
    assert a.shape == (_N, _D) and b.shape == (_M, _D)
    norm = bool(int(np.asarray(normalize)))

    nc = _get_nc(norm)

    ad = [_pack_ad(a[ni * _AN:(ni + 1) * _AN]) for ni in range(_GRID_N)]
    aa = [_pack_aa(a[ni * _AN:(ni + 1) * _AN]) for ni in range(_GRID_N)]
    bd = [_pack_bd(b[mj * _BM:(mj + 1) * _BM]) for mj in range(_GRID_M)]

    in_maps = []
    for c in range(_GRID_N * _GRID_M):
        ni, mj = divmod(c, _GRID_M)
        in_maps.append({"ad": ad[ni], "aa": aa[ni], "bd": bd[mj]})

    res = run_bass_kernel_spmd(
        nc, in_maps, core_ids=list(range(_GRID_N * _GRID_M)), trace=TRACE
    )
    LAST_RESULTS = res

    out = np.empty((_N, _M), dtype=np.float32)
    for c in range(_GRID_N * _GRID_M):
        ni, mj = divmod(c, _GRID_M)
        out[ni * _AN:(ni + 1) * _AN, mj * _BM:(mj + 1) * _BM] = \
            res.results[c]["out"].astype(np.float32)
    return out


# revision 32
# speedup vs baseline: 1.0130x; 1.0130x over previous
"""Cosine-similarity (pairwise, normalized by sqrt(|a||b|)+eps) Trainium2 kernel.

Problem: first_vector [8192, 512] f32, second_vector [8192, 512] f32,
output sim [8192, 8192] f32 with
    sim = (A @ B.T) / (sqrt(|A_n| * |B_m|) + 1e-6)        (normalize=1)

Strategy (8 NeuronCores, SPMD, no collectives):
  * 2D shard: 4-way over A rows x 2-way over B rows. Core c=(ni,mj)
    computes the [2048, 4096] output slab at (ni*2048, mj*4096).
  * All matmul operands are packed HOST-side into d-major (transposed)
    tiled layouts, so the PE does nothing but the 512 GEMM matmuls
    (215.8ns each at the fp16 roofline, measured) plus 8 cheap
    sum-of-squares matmuls. The previous design transposed on-device
    via identity matmuls (+15us PE) and burned ~55us of DVE/ACT on
    transpose evacuations + operand scaling.
  * Normalization is separable (eps shifts the result by <1e-7 rel):
    scale A rows by ssqA^-1/4 and B rows by ssqB^-1/4.
      - A ssq: an auxiliary ROW-major copy of A is loaded (+2MB DMA);
        square-with-accum_out gives ssq in column form [128,1] per
        row-tile - exactly the per-partition scalar an evacuation wants.
      - B ssq: square+add the d-major tiles, then one matmul against an
        all-ones [128,128] lhsT: out[p,f] = sum_k s1[k,f] for every p -
        reduce AND broadcast in one ~215ns PE op (the probed gpsimd
        partition_all_reduce costs 3537ns).
      - chains run reciprocal_approx_fast (DVE, ~51 ULP, straight off
        PSUM for B) then sqrt+sqrt on ACT, landing sbB in f16
        broadcast form / saT in f32 column form.
  * Evacuations (64 x [128,2,512] PSUM f32 -> f16) all run as ONE DVE
    scalar_tensor_tensor per slot on RAW B operands:
    out = (psum * saT[t]) * sbB[2p:2p+2] (~1300ns, barely above a plain
    cast). No operand pre-scaling exists at all, so no B-scale work ever
    gates a GEMM slot, and GpSimd multiplies never contend with PE SBUF
    reads (pre-scaling B on GpSimd measured SLOWER end-to-end).
    Evacuations are emitted 2 slots behind the GEMM so chain producers
    precede consumers in each engine's stream; PSUM holds 3 slots + the
    ssq/warmup bank.
  * The first four slots are single-B-group (t-pair x one s): their
    evacuations depend on only ONE sbB chain (b0's lands ~4.5us before
    b1's) which beats the PSUM-recycle deadline while bd1 still streams.
  * Startup is DMA-stream-bound (~0.27MB/us effective): transfers are
    batched (each dma_start costs ~0.6us of Sync issue time) and
    ordered by need-time; nothing issues until the ~6us instruction
    -load prolog ends (excluded from the reported exec time).
  * The PE clock governor is ACTIVITY-driven: 16 back-to-back warm-up
    matmuls during the input-DMA wait pull the 0.65 -> 2.4GHz ramp
    completion from ~28us down to ~12us (measured via HAM track).
    Note the device can also pin itself to 2.0GHz under sustained
    benching load (259ns steady matmuls instead of 215.8ns) - run-to-
    run variance of several us is thermal, not structural.
  * The ~9.5us exit protocol (289 EVENT_SEMAPHORE barrier events) and
    ~6us instruction-load prolog are fixed costs, identical even for a
    5-op probe kernel.
  * fp16 everywhere off-chip: 8MB in, 16.8MB out per core, against a
    ~112us PE window - DMA never binds after the startup phase.
"""

import numpy as np

_N, _M, _D = 8192, 8192, 512
_P = 128
_GRID_N, _GRID_M = 4, 2
_AN = _N // _GRID_N        # A rows per core (2048)
_BM = _M // _GRID_M        # B rows per core (4096)
_KC = _D // _P             # contraction chunks (4)
_NS = 512                  # moving free dim per matmul (one PSUM bank of f32)

TRACE = False              # test harness sets True to collect an NTFF profile
LAST_RESULTS = None        # BassKernelResults of the last run (for test.py)

_NC_CACHE = {}


def _build_nc(normalize: bool):
    import concourse.bass as bass
    import concourse.mybir as mybir
    import concourse.tile as tile
    from concourse import bacc

    f32 = mybir.dt.float32
    f16 = mybir.dt.float16
    nc = bacc.Bacc("TRN2", target_bir_lowering=False, debug=False,
                   enable_asserts=False)

    KA = _AN // _P             # 16 A row-tiles (8 pairs)
    NSC = _BM // _NS           # 8 B column groups of 512
    NG = KA // 2               # 8 A pairs
    SQ = mybir.ActivationFunctionType.Square
    CP = mybir.ActivationFunctionType.Copy
    MUL = mybir.AluOpType.mult
    ADD = mybir.AluOpType.add

    # d-major A pairs: row (g*128+p), col (h, kc, r) = A[256g+128h+r, 128kc+p]
    ad_d = nc.declare_dram_parameter("ad", [NG * _P, 2 * _KC * _P], f16,
                                     isOutput=False)
    # d-major B groups: row (s*128+p), col (kc, c) = B[512s+c, 128kc+p]
    bd_d = nc.declare_dram_parameter("bd", [NSC * _P, _KC * _NS], f16,
                                     isOutput=False)
    # row-major A pairs (aux, for ssq): row (g*128+p), col (h, d)
    aa_d = nc.declare_dram_parameter("aa", [NG * _P, 2 * _D], f16,
                                     isOutput=False)
    out_d = nc.declare_dram_parameter("out", [_AN, _BM], f16, isOutput=True)

    # Greedy DVE/ACT balance for the sa-only evacuations (ns, measured).
    load = {"dve": 0.0, "act": 0.0}

    with tile.TileContext(nc) as tc:
        with (
            tc.tile_pool(name="const", bufs=1) as const_pool,
            tc.tile_pool(name="persist", bufs=1) as persist,
            tc.tile_pool(name="sqp", bufs=2) as sqp,
            tc.tile_pool(name="s1p", bufs=2) as s1p,
            tc.tile_pool(name="chp", bufs=2) as chp,
            tc.tile_pool(name="dmp", bufs=2) as dmp,
            tc.tile_pool(name="ostp", bufs=8) as ostp,
            tc.tile_pool(name="mpsum", bufs=3, space=bass.MemorySpace.PSUM) as mpsum,
            tc.tile_pool(name="ssqp", bufs=2, space=bass.MemorySpace.PSUM) as ssqp,
        ):
            # Memsets first so every engine's first data arrives ASAP.
            wsrc = const_pool.tile([_P, _NS], f16)
            nc.vector.memset(wsrc[:], 0.5)
            ones = const_pool.tile([_P, _P], f16)
            nc.vector.memset(ones[:], 1.0)
            dsrc = const_pool.tile([_P, 1], f32)
            nc.vector.memset(dsrc[:], 1.0)

            # ACT table preloads on dependency-free data (Sqrt is needed
            # first, by the chains; Square by the A/B squares).
            ddst = const_pool.tile([_P, 1], f32)
            nc.scalar.sqrt(ddst[:], dsrc[:])
            nc.scalar.activation(ddst[:], dsrc[:], SQ)

            # Consolidated operand tiles (single allocations; sub-tile
            # DMAs + sub-tile reads use Tile's range-based deps).
            adT = persist.tile([_P, NG, 2, _KC, _P], f16, name="adT", tag="adT")
            bdT = persist.tile([_P, NSC, _KC, _NS], f16, name="bdT", tag="bdT")
            aaT = persist.tile([_P, NG, 2, _D], f16, name="aaT", tag="aaT")

            ssqA = persist.tile([_P, KA], f32, name="ssqA", tag="ssqA")
            shA = persist.tile([_P, KA], f32, name="shA", tag="shA")
            saT = persist.tile([_P, KA], f32, name="saT", tag="saT")
            sbB = persist.tile([_P, NSC, _NS], f16, name="sbB", tag="sbB")

            # Input DMA: each dma_start costs ~0.6us of Sync-sequencer
            # issue time, so batch multi-pair transfers; first-needed
            # tiles go individually, in need order.
            def dma_bd(s0, s1_):
                n = s1_ - s0
                nc.sync.dma_start(
                    bdT[:, s0:s1_],
                    bd_d[s0 * _P:s1_ * _P, :]
                    .rearrange("(s p) (k c) -> p s k c", s=n, k=_KC))

            def dma_ad(g0, g1):
                n = g1 - g0
                nc.sync.dma_start(
                    adT[:, g0:g1],
                    ad_d[g0 * _P:g1 * _P, :]
                    .rearrange("(g p) (h k r) -> p g h k r", g=n, h=2, k=_KC))

            def dma_aa(g0, g1):
                n = g1 - g0
                nc.sync.dma_start(
                    aaT[:, g0:g1],
                    aa_d[g0 * _P:g1 * _P, :]
                    .rearrange("(g p) (h d) -> p g h d", g=n, h=2))

            # Arrival model: transfers stream at ~0.28MB/us starting ~2us
            # after the prolog; order by need-time (first evac needs aa01;
            # the PE ramp makes early GEMM slots ~2x long).
            dma_bd(0, 1)
            dma_ad(0, 1)
            dma_aa(0, 2)
            dma_ad(1, 2)
            dma_bd(1, 2)
            dma_ad(2, 4)
            dma_aa(2, 4)
            dma_aa(4, 6)
            dma_bd(2, 4)
            dma_aa(6, 8)
            dma_ad(4, 8)
            dma_bd(4, 8)

            # Warm the PE clock during the input-DMA wait (~5us until bd0
            # and ad0 land). The clock governor appears activity-driven:
            # runs whose PE idles early reach the full 2.4GHz pstate
            # several us later, so keep the PE busy back-to-back.
            for _ in range(8):
                warm = ssqp.tile([_P, _NS], f32, tag="ssqp")
                for _ in range(2):
                    nc.tensor.matmul(warm[:], lhsT=ones[:], rhs=wsrc[:],
                                     start=True, stop=True)

            def b_squares(s, engine, adds):
                """Squares + adds of d-major B group s -> s1 [128, 512]."""
                sq = sqp.tile([_P, _KC, _NS], f16, tag="sq")
                for j in range(2):
                    if engine == "dve":
                        nc.vector.tensor_tensor(sq[:, 2 * j:2 * j + 2],
                                                bdT[:, s, 2 * j:2 * j + 2],
                                                bdT[:, s, 2 * j:2 * j + 2],
                                                op=MUL)
                        load["dve"] += 1380
                    else:
                        nc.scalar.activation(sq[:, 2 * j:2 * j + 2],
                                             bdT[:, s, 2 * j:2 * j + 2], SQ)
                        load["act"] += 1060
                s2 = s1p.tile([_P, 2, _NS], f16, tag="s2")
                s1 = s1p.tile([_P, _NS], f16, tag="s1")
                eng = nc.vector if adds == "dve" else nc.gpsimd
                eng.tensor_tensor(s2[:], sq[:, 0:2], sq[:, 2:4], op=ADD)
                eng.tensor_tensor(s1[:], s2[:, 0], s2[:, 1], op=ADD)
                if adds == "dve":
                    load["dve"] += 1900
                return s1

            def b_ssq_mm(s, s1):
                """All-ones matmul: ssq_b reduced + broadcast into PSUM."""
                ps = ssqp.tile([_P, _NS], f32, tag="ssqp")
                nc.tensor.matmul(ps[:], lhsT=ones[:], rhs=s1[:], start=True,
                                 stop=True)
                return ps

            def b_chain(s, ps):
                """sbB[s] = ssq^-1/4: reciprocal_approx_fast (DVE, reads
                PSUM) then sqrt+sqrt (ACT, f16 out)."""
                rec = chp.tile([_P, _NS], f32, tag="recb")
                nc.vector.reciprocal_approx_fast(rec[:], ps[:])
                nc.scalar.sqrt(rec[:], rec[:])
                nc.scalar.sqrt(sbB[:, s, :], rec[:])
                load["dve"] += 800
                load["act"] += 1440

            def a_ssq(g, engine):
                """ssq of A pair g (column form) + chain into saT."""
                for h in range(2):
                    t = 2 * g + h
                    dump = dmp.tile([_P, _D], f16, tag="dump")
                    if engine == "dve":
                        nc.vector.scalar_tensor_tensor(
                            dump[:], aaT[:, g, h], 1.0, aaT[:, g, h],
                            op0=MUL, op1=MUL, accum_out=ssqA[:, t:t + 1])
                        load["dve"] += 700
                    else:
                        nc.scalar.activation(dump[:], aaT[:, g, h], SQ,
                                             accum_out=ssqA[:, t:t + 1])
                        load["act"] += 800
                c = slice(2 * g, 2 * g + 2)
                nc.scalar.sqrt(shA[:, c], ssqA[:, c])
                nc.scalar.sqrt(shA[:, c], shA[:, c])
                nc.vector.reciprocal(saT[:, c], shA[:, c])
                load["act"] += 400
                load["dve"] += 120

            # Pre-loop prep: B0/B1 squares on DVE (startup critical);
            # A pair 0's ACT half fills ACT's idle window before the B
            # chains' sqrts queue up.
            if normalize:
                s1_b = {0: b_squares(0, "dve", "dve"),
                        1: b_squares(1, "dve", "dve")}
                ps_b = {}
                a_ssq(0, "dve")

            # Deferred-evacuation pipeline: evac for slot i is emitted
            # alongside slot i+2 so chain producers precede consumers in
            # every engine's program order. mpsum holds 3 slots.
            pending = []

            def emit_evac():
                kind, p, t, ps2 = pending.pop(0)
                ost = ostp.tile([_P, 2, _NS], f16, tag="ost")
                if kind == "split":
                    # Single-B-group startup slot: bank h holds row-tile
                    # (2p... here p is the t-pair index) x group t(=s).
                    tp, s = p, t
                    for h in range(2):
                        tt = 2 * tp + h
                        nc.vector.scalar_tensor_tensor(
                            ost[:, h], ps2[:, h], saT[:, tt:tt + 1],
                            sbB[:, s, :], op0=MUL, op1=MUL)
                        load["dve"] += 700
                        nc.sync.dma_start(
                            out_d[tt * _P:(tt + 1) * _P,
                                  s * _NS:(s + 1) * _NS],
                            ost[:, h],
                        )
                    return
                if not normalize:
                    if load["dve"] <= load["act"]:
                        nc.vector.tensor_copy(ost[:], ps2[:])
                        load["dve"] += 1230
                    else:
                        nc.scalar.copy(ost[:], ps2[:])
                        load["act"] += 1100
                elif p == 0:
                    # Raw B operands; both scales in one DVE op.
                    nc.vector.scalar_tensor_tensor(
                        ost[:], ps2[:], saT[:, t:t + 1], sbB[:, 0:2, :],
                        op0=MUL, op1=MUL)
                    load["dve"] += 1300
                else:
                    nc.vector.scalar_tensor_tensor(
                        ost[:], ps2[:], saT[:, t:t + 1],
                        sbB[:, 2 * p:2 * p + 2, :], op0=MUL, op1=MUL)
                    load["dve"] += 1300
                nc.sync.dma_start(
                    out_d[t * _P:(t + 1) * _P,
                          2 * p * _NS:(2 * p + 2) * _NS],
                    ost[:].rearrange("m h f -> m (h f)"),
                )

            # Per-slot prep emissions for p=0. The PE ramp makes early
            # slots ~2x longer, absorbing the chain latencies; A-pair
            # ssq goes late in the ACT queue (only needed by evac t=2g).
            def prep_p0(t):
                if t == 0:
                    # Both ones-matmuls up front: slot 1's 8 GEMM matmuls
                    # must not precede ones_mm(1) in PE priority order, or
                    # the B1 chain (and the first evacuation) slips ~3us.
                    ps_b[0] = b_ssq_mm(0, s1_b[0])
                    b_chain(0, ps_b[0])
                    ps_b[1] = b_ssq_mm(1, s1_b[1])
                    b_chain(1, ps_b[1])
                elif t == 2:
                    a_ssq(1, "dve")
                elif t == 3:
                    a_ssq(2, "act")
                elif t == 7:
                    a_ssq(3, "act")
                elif 8 <= t <= 11:
                    a_ssq(t - 4, "act")    # pairs 4..7
                elif t == 12:
                    # B2/B3 prep late: emitting its big ACT squares any
                    # earlier lets them cut ahead of the A-pair ssq ops on
                    # ACT (saT for the t>=12 evacuations). The chain tail
                    # (chain_b(3) at p1.t0) MUST still be emitted before
                    # the first p=1 evacuation (p1.t2) or that stt reads
                    # sbB[:,3] before ACT writes it.
                    s1_b[2] = b_squares(2, "act", "pool")
                elif t == 13:
                    s1_b[3] = b_squares(3, "act", "pool")
                elif t == 14:
                    ps_b[2] = b_ssq_mm(2, s1_b[2])
                elif t == 15:
                    ps_b[3] = b_ssq_mm(3, s1_b[3])
                    b_chain(2, ps_b[2])

            def prep_p(p, t):
                sa_, sb_ = 2 * p + 2, 2 * p + 3
                if t == 0 and p == 1:
                    b_chain(3, ps_b[3])
                if t == 2:
                    s1_b[sa_] = b_squares(sa_, "act", "pool")
                elif t == 3:
                    s1_b[sb_] = b_squares(sb_, "act", "pool")
                elif t == 5:
                    ps_b[sa_] = b_ssq_mm(sa_, s1_b[sa_])
                elif t == 6:
                    ps_b[sb_] = b_ssq_mm(sb_, s1_b[sb_])
                elif t == 7:
                    b_chain(sa_, ps_b[sa_])
                elif t == 8:
                    b_chain(sb_, ps_b[sb_])

            # GEMM stream: column-group pairs p, row-tiles t. When
            # normalizing, p=0's first four slots are single-B-group
            # (t-pair x one s): their evacuations need only ONE sbB chain
            # and the PE starts on bd0/ad0/ad1 while bd1 still streams -
            # b0's chain lands ~4.5us before b1's.
            def mm_slot(tlist_s):
                ps2 = mpsum.tile([_P, 2, _NS], f32, tag="ps2")
                for h, (t, s) in enumerate(tlist_s):
                    for k in range(_KC):
                        rhs = bdT[:, s, k, :]
                        nc.tensor.matmul(
                            ps2[:, h],
                            lhsT=adT[:, t // 2, t % 2, k, :],
                            rhs=rhs,
                            start=(k == 0),
                            stop=(k == _KC - 1),
                        )
                return ps2

            for p in range(NSC // 2):
                split_head = normalize and p == 0
                for t in range(KA):
                    if len(pending) >= 2:
                        emit_evac()

                    if split_head and t < 4:
                        tp, s = t % 2, t // 2
                        ps2 = mm_slot([(2 * tp, s), (2 * tp + 1, s)])
                        pending.append(("split", tp, s, ps2))
                    else:
                        ps2 = mm_slot([(t, 2 * p), (t, 2 * p + 1)])
                        pending.append(("pair", p, t, ps2))

                    if normalize:
                        if p == 0:
                            prep_p0(t)
                        elif p < 3:
                            prep_p(p, t)

            while pending:
                emit_evac()

    nc.compile()
    return nc


def _get_nc(normalize: bool):
    key = bool(normalize)
    if key not in _NC_CACHE:
        _NC_CACHE[key] = _build_nc(key)
    return _NC_CACHE[key]


def _pack_ad(a16):
    """[2048, 512] f16 -> d-major pair tiles [1024, 1024]:
    row (g*128+p), col (h*512 + kc*128 + r) = A[g*256+h*128+r, kc*128+p]."""
    return (a16.reshape(8, 2, _P, _KC, _P)
            .transpose(0, 4, 1, 3, 2)
            .reshape(8 * _P, 2 * _KC * _P))


def _pack_bd(b16):
    """[4096, 512] f16 -> d-major group tiles [1024, 2048]:
    row (s*128+p), col (kc*512 + c) = B[s*512+c, kc*128+p]."""
    return (b16.reshape(8, _NS, _KC, _P)
            .transpose(0, 3, 2, 1)
            .reshape(8 * _P, _KC * _NS))


def _pack_aa(a16):
    """[2048, 512] f16 row-major pairs [1024, 1024]: partition p of pair g
    holds rows g*256+p and g*256+128+p side by side."""
    return (a16.reshape(8, 2, _P, _D)
            .transpose(0, 2, 1, 3)
            .reshape(8 * _P, 2 * _D))


def kernel(first_vector, second_vector, normalize):
    global LAST_RESULTS
    from concourse.bass_utils import run_bass_kernel_spmd

    a = np.asarray(first_vector, dtype=np.float32).astype(np.float16)
    b = np.asarray(second_vector, dtype=np.float32).astype(np.float16)
    assert a.shape == (_N, _D) and b.shape == (_M, _D)
    norm = bool(int(np.asarray(normalize)))

    nc = _get_nc(norm)

    ad = [_pack_ad(a[ni * _AN:(ni + 1) * _AN]) for ni in range(_GRID_N)]
    aa = [_pack_aa(a[ni * _AN:(ni + 1) * _AN]) for ni in range(_GRID_N)]
    bd = [_pack_bd(b[mj * _BM:(mj + 1) * _BM]) for mj in range(_GRID_M)]

    in_maps = []
    for c in range(_GRID_N * _GRID_M):
        ni, mj = divmod(c, _GRID_M)
        in_maps.append({"ad": ad[ni], "aa": aa[ni], "bd": bd[mj]})

    res = run_bass_kernel_spmd(
        nc, in_maps, core_ids=list(range(_GRID_N * _GRID_M)), trace=TRACE
    )
    LAST_RESULTS = res

    out = np.empty((_N, _M), dtype=np.float32)
    for c in range(_GRID_N * _GRID_M):
        ni, mj = divmod(c, _GRID_M)
        out[ni * _AN:(ni + 1) * _AN, mj * _BM:(mj + 1) * _BM] = \
            res.results[c]["out"].astype(np.float32)
    return out
